# revision 6
# baseline (speedup 1.0000x reference)
"""Trainium2 Bass kernel for the LDE guided-attention module.

Sharding: 8 cores = 2 samples x 4 row-quarters of the N=9216 attention rows.
Zero cross-core communication: each core redundantly computes the (cheap)
conv trunk for its sample, then its quarter of the softmax(d1@d2)@c1 rows
flash-attention style -- the [N,N] map never leaves PSUM/SBUF.

Layouts (per core, sample s=core//4, quarter q=core%4):
  - trunk conv3x3 as 9 offset-matmuls over a zero-padded [64, 98, 98] slab
  - d2   [32, 9216] channel-major (lhsT tiles for scores)
  - c1aug [128, 72, 33] = c1 in N-major layout + ones column (fused rowsum)
  - d1q  [32, 2304], d0q [64, 2304] from a 26-row halo slab of depth
  - scores S^T tile [128, Rb] = matmul(lhsT=d2_tile, rhs=d1q_blk); exp on ACT;
    guided^T+rowsum accumulate via matmul(lhsT=c1aug_tile, rhs=expS)
  - epilogue: out = (wch4 @ guided^T) * (1/rowsum) + d0q
All matmul operands bitcast to float32r: full fp32 data at 1 cycle/row.
"""

import sys

for _p in ("/opt/trn_rl_repo",):
    if _p not in sys.path:
        sys.path.insert(0, _p)

import numpy as np

import concourse.bass as bass
import concourse.bacc as bacc
import concourse.mybir as mybir
from concourse import tile
from concourse.bass_utils import run_bass_kernel_spmd

F32 = mybir.dt.float32
F32R = mybir.dt.float32r
AF = mybir.ActivationFunctionType

C = 64          # channels
CQ = 32         # C // 2
H = W = 96
N = H * W       # 9216
NT = N // 128   # 72 column tiles
QROWS = 24      # image rows per quarter
NQ = QROWS * W  # 2304 attention rows per core
PW = 98         # padded width
CHUNK_ROWS = 4
CHUNK = CHUNK_ROWS * W  # 384
BLOCKS = [(0, 512), (512, 512), (1024, 512), (1536, 512), (2048, 256)]

_cache = {}


def _r(ap):
    return ap


def _trunk_chunk(nc, tc, kpool, ps, slab, row0, w1t_sb, w2t_sb, b1_sb, b2_sb,
                 a1, a2, out_ap=None):
    """conv3x3+PReLU then conv1x1+PReLU for 4 image rows starting at
    slab row row0 (slab has 1 halo row on top). Returns [64, 384] AP."""
    psc = ps.tile([C, CHUNK], F32, tag="pscv")
    for k in range(9):
        ky, kx = divmod(k, 3)
        rhs = slab[:, row0 + ky: row0 + ky + CHUNK_ROWS, kx: kx + W]
        nc.tensor.matmul(psc[:], _r(w1t_sb[:, k * C:(k + 1) * C]), _r(rhs),
                         start=(k == 0), stop=(k == 8))
    # prelu(y+b, a) = a*(y+b) + (1-a)*relu(y+b); relu(s*y+s*b) = s*relu(y+b)
    r = kpool.tile([C, CHUNK], F32, tag="tr")
    t = kpool.tile([C, CHUNK], F32, tag="tt")
    nc.scalar.activation(r[:], psc[:], AF.Relu, bias=b1_sb[:, 1:2], scale=1.0 - a1)
    nc.scalar.activation(t[:], psc[:], AF.Identity, bias=b1_sb[:, 0:1], scale=a1)
    pre = kpool.tile([C, CHUNK], F32, tag="tp")
    nc.vector.tensor_add(pre[:], r[:], t[:])

    psc2 = ps.tile([C, CHUNK], F32, tag="pscv")
    nc.tensor.matmul(psc2[:], _r(w2t_sb[:]), _r(pre[:]), start=True, stop=True)
    r2 = kpool.tile([C, CHUNK], F32, tag="tr")
    t2 = kpool.tile([C, CHUNK], F32, tag="tt")
    nc.scalar.activation(r2[:], psc2[:], AF.Relu, bias=b2_sb[:, 1:2], scale=1.0 - a2)
    nc.scalar.activation(t2[:], psc2[:], AF.Identity, bias=b2_sb[:, 0:1], scale=a2)
    if out_ap is None:
        c = kpool.tile([C, CHUNK], F32, tag="tc")
        out_ap = c[:]
    nc.vector.tensor_add(out_ap, r2[:], t2[:])
    return out_ap


def _build(a1: float, a2: float):
    nc = bacc.Bacc(None, target_bir_lowering=False)
    xr = nc.declare_dram_parameter("xr", [C, N], F32, isOutput=False)
    xd = nc.declare_dram_parameter("xd", [C, N], F32, isOutput=False)
    xdq = nc.declare_dram_parameter("xdq", [C, 26 * W], F32, isOutput=False)
    w1t = nc.declare_dram_parameter("w1t", [C, 9 * C], F32, isOutput=False)
    w2t = nc.declare_dram_parameter("w2t", [C, C], F32, isOutput=False)
    wch1t = nc.declare_dram_parameter("wch1t", [C, CQ], F32, isOutput=False)
    wch2t = nc.declare_dram_parameter("wch2t", [C, CQ], F32, isOutput=False)
    wch3t = nc.declare_dram_parameter("wch3t", [C, CQ], F32, isOutput=False)
    wch4t = nc.declare_dram_parameter("wch4t", [CQ, C], F32, isOutput=False)
    b1 = nc.declare_dram_parameter("b1", [C, 2], F32, isOutput=False)
    b2 = nc.declare_dram_parameter("b2", [C, 2], F32, isOutput=False)
    out = nc.declare_dram_parameter("out", [C, NQ], F32, isOutput=True)

    with tile.TileContext(nc) as tc:
        with (
            tc.tile_pool(name="const", bufs=1) as cpool,
            tc.tile_pool(name="xpad", bufs=1) as xpool,
            tc.tile_pool(name="big", bufs=1) as bpool,
            tc.tile_pool(name="chunk", bufs=3) as kpool,
            tc.tile_pool(name="pt", bufs=3) as ptpool,
            tc.tile_pool(name="ep", bufs=2) as eppool,
            tc.tile_pool(name="ps_s", bufs=2, space="PSUM") as ps_s,
            tc.tile_pool(name="ps_g", bufs=2, space="PSUM") as ps_g,
            tc.tile_pool(name="ps_m", bufs=3, space="PSUM") as ps_m,
        ):
            # ---- constants ----
            w1t_sb = cpool.tile([C, 9 * C], F32)
            nc.sync.dma_start(w1t_sb[:], w1t[:])
            w2t_sb = cpool.tile([C, C], F32)
            nc.sync.dma_start(w2t_sb[:], w2t[:])
            wch1t_sb = cpool.tile([C, CQ], F32)
            nc.sync.dma_start(wch1t_sb[:], wch1t[:])
            wch2t_sb = cpool.tile([C, CQ], F32)
            nc.sync.dma_start(wch2t_sb[:], wch2t[:])
            wch3t_sb = cpool.tile([C, CQ], F32)
            nc.sync.dma_start(wch3t_sb[:], wch3t[:])
            wch4t_sb = cpool.tile([CQ, C], F32)
            nc.sync.dma_start(wch4t_sb[:], wch4t[:])
            b1_sb = cpool.tile([C, 2], F32)
            nc.sync.dma_start(b1_sb[:], b1[:])
            b2_sb = cpool.tile([C, 2], F32)
            nc.sync.dma_start(b2_sb[:], b2[:])
            ones_sb = cpool.tile([1, C], F32)
            nc.vector.memset(ones_sb[:], 1.0)

            # ---- persistent intermediates ----
            d2_sb = bpool.tile([CQ, N], F32)           # scores lhsT source
            c1aug = bpool.tile([128, NT, CQ + 1], F32)  # c1 N-major + ones col
            d1q = bpool.tile([CQ, NQ], F32)
            d0q = bpool.tile([C, NQ], F32)
            nc.vector.memset(c1aug[:, :, CQ:CQ + 1], 1.0)

            # ---- depth quarter (halo slab): d0q, d1q ----
            dq_slab = xpool.tile([C, 26, PW], F32, tag="dqslab")
            nc.vector.memset(dq_slab[:, :, 0:1], 0.0)
            nc.vector.memset(dq_slab[:, :, PW - 1:PW], 0.0)
            nc.sync.dma_start(
                dq_slab[:, :, 1:W + 1],
                xdq[:].rearrange("c (r w) -> c r w", w=W),
            )
            for j in range(NQ // CHUNK):
                sl = slice(j * CHUNK, (j + 1) * CHUNK)
                _trunk_chunk(nc, tc, kpool, ps_m, dq_slab, 4 * j, w1t_sb,
                             w2t_sb, b1_sb, b2_sb, a1, a2, out_ap=d0q[:, sl])
                psq = ps_m.tile([CQ, CHUNK], F32, tag="pscv")
                nc.tensor.matmul(psq[:], _r(wch2t_sb[:]), _r(d0q[:, sl]),
                                 start=True, stop=True)
                nc.vector.tensor_copy(d1q[:, sl], psq[:])

            # ---- rgb trunk -> c1aug (N-major) ----
            slab = xpool.tile([C, PW, PW], F32, tag="slab")
            for edge in (0, PW - 1):
                nc.vector.memset(slab[:, edge, :], 0.0)
                nc.vector.memset(slab[:, 1:PW - 1, edge:edge + 1], 0.0)
            xr_r = xr[:].rearrange("c (h w) -> c h w", w=W)
            for piece in range(4):
                rs = slice(piece * QROWS, (piece + 1) * QROWS)
                nc.sync.dma_start(slab[:, 1 + piece * QROWS:1 + (piece + 1) * QROWS, 1:W + 1],
                                  xr_r[:, rs, :])
            for j in range(N // CHUNK):
                c = _trunk_chunk(nc, tc, kpool, ps_m, slab, 4 * j, w1t_sb,
                                 w2t_sb, b1_sb, b2_sb, a1, a2)
                for i in range(3):
                    ti = 3 * j + i
                    psn = ps_m.tile([128, CQ], F32, tag="pscv")
                    nc.tensor.matmul(psn[:], _r(c[:, i * 128:(i + 1) * 128]),
                                     _r(wch1t_sb[:]), start=True, stop=True)
                    nc.vector.tensor_copy(c1aug[:, ti, 0:CQ], psn[:])

            # ---- depth trunk -> d2 (channel-major) ----
            slab2 = xpool.tile([C, PW, PW], F32, tag="slab")
            for edge in (0, PW - 1):
                nc.vector.memset(slab2[:, edge, :], 0.0)
                nc.vector.memset(slab2[:, 1:PW - 1, edge:edge + 1], 0.0)
            xd_r = xd[:].rearrange("c (h w) -> c h w", w=W)
            for piece in range(4):
                rs = slice(piece * QROWS, (piece + 1) * QROWS)
                nc.sync.dma_start(slab2[:, 1 + piece * QROWS:1 + (piece + 1) * QROWS, 1:W + 1],
                                  xd_r[:, rs, :])
            for j in range(N // CHUNK):
                d = _trunk_chunk(nc, tc, kpool, ps_m, slab2, 4 * j, w1t_sb,
                                 w2t_sb, b1_sb, b2_sb, a1, a2)
                psd = ps_m.tile([CQ, CHUNK], F32, tag="pscv")
                nc.tensor.matmul(psd[:], _r(wch3t_sb[:]), _r(d),
                                 start=True, stop=True)
                nc.vector.tensor_copy(d2_sb[:, j * CHUNK:(j + 1) * CHUNK], psd[:])

            # ---- streaming attention over row blocks ----
            for (o, rb) in BLOCKS:
                ps_acc = ps_g.tile([CQ + 1, rb], F32, tag="psg")
                for t in range(NT):
                    ps_sc = ps_s.tile([128, rb], F32, tag="pss")
                    nc.tensor.matmul(ps_sc[:], _r(d2_sb[:, t * 128:(t + 1) * 128]),
                                     _r(d1q[:, o:o + rb]), start=True, stop=True)
                    pT = ptpool.tile([128, rb], F32, tag="pt")
                    nc.scalar.activation(pT[:], ps_sc[:], AF.Exp)
                    nc.tensor.matmul(ps_acc[:], _r(c1aug[:, t, :]), _r(pT[:]),
                                     start=(t == 0), stop=(t == NT - 1),
                                     skip_group_check=True)
                g_sb = eppool.tile([CQ, rb], F32, tag="gsb")
                nc.vector.tensor_copy(g_sb[:], ps_acc[0:CQ, :])
                sum_sb = eppool.tile([1, rb], F32, tag="ssb")
                nc.vector.tensor_copy(sum_sb[:], ps_acc[CQ:CQ + 1, :])
                ps_b = ps_m.tile([C, rb], F32, tag="pscv")
                nc.tensor.matmul(ps_b[:], _r(ones_sb[:]), _r(sum_sb[:]),
                                 start=True, stop=True)
                rcp = eppool.tile([C, rb], F32, tag="rcp")
                nc.vector.reciprocal(rcp[:], ps_b[:])
                ps_o = ps_m.tile([C, rb], F32, tag="pscv")
                nc.tensor.matmul(ps_o[:], _r(wch4t_sb[:]), _r(g_sb[:]),
                                 start=True, stop=True)
                o1 = eppool.tile([C, rb], F32, tag="o1")
                nc.vector.tensor_mul(o1[:], ps_o[:], rcp[:])
                osb = eppool.tile([C, rb], F32, tag="osb")
                nc.vector.tensor_add(osb[:], o1[:], d0q[:, o:o + rb])
                nc.sync.dma_start(out[:, o:o + rb], osb[:])

    nc.finalize()
    return nc


def _prep_inputs(rgb, depth, w1, b1, a1, w2, b2, a2, wch1, wch2, wch3, wch4):
    rgb = np.asarray(rgb, np.float32)
    depth = np.asarray(depth, np.float32)
    # w1t[ci, (ky*3+kx)*C + co]
    w1t = np.ascontiguousarray(
        np.transpose(np.asarray(w1, np.float32), (1, 2, 3, 0)).reshape(C, 9 * C))
    w2t = np.ascontiguousarray(np.asarray(w2, np.float32)[:, :, 0, 0].T)
    wch1t = np.ascontiguousarray(np.asarray(wch1, np.float32)[:, :, 0, 0].T)
    wch2t = np.ascontiguousarray(np.asarray(wch2, np.float32)[:, :, 0, 0].T)
    wch3t = np.ascontiguousarray(np.asarray(wch3, np.float32)[:, :, 0, 0].T)
    wch4t = np.ascontiguousarray(np.asarray(wch4, np.float32)[:, :, 0, 0].T)
    a1f = float(np.asarray(a1)); a2f = float(np.asarray(a2))
    b1a = np.stack([a1f * np.asarray(b1, np.float32),
                    (1.0 - a1f) * np.asarray(b1, np.float32)], axis=1)
    b2a = np.stack([a2f * np.asarray(b2, np.float32),
                    (1.0 - a2f) * np.asarray(b2, np.float32)], axis=1)

    in_maps = []
    for core in range(8):
        s, q = divmod(core, 4)
        xdq = np.zeros((C, 26, W), np.float32)
        for r_slab in range(26):
            r_img = q * QROWS - 1 + r_slab
            if 0 <= r_img < H:
                xdq[:, r_slab, :] = depth[s, :, r_img, :]
        in_maps.append({
            "xr": np.ascontiguousarray(rgb[s].reshape(C, N)),
            "xd": np.ascontiguousarray(depth[s].reshape(C, N)),
            "xdq": np.ascontiguousarray(xdq.reshape(C, 26 * W)),
            "w1t": w1t, "w2t": w2t,
            "wch1t": wch1t, "wch2t": wch2t, "wch3t": wch3t, "wch4t": wch4t,
            "b1": np.ascontiguousarray(b1a), "b2": np.ascontiguousarray(b2a),
        })
    return in_maps, (a1f, a2f)


def kernel(rgb, depth, w1, b1, a1, w2, b2, a2, wch1, wch2, wch3, wch4,
           **run_kwargs):
    in_maps, (a1f, a2f) = _prep_inputs(rgb, depth, w1, b1, a1, w2, b2, a2,
                                       wch1, wch2, wch3, wch4)
    key = (a1f, a2f)
    if key not in _cache:
        _cache[key] = _build(a1f, a2f)
    nc = _cache[key]
    res = run_bass_kernel_spmd(nc, in_maps, list(range(8)), **run_kwargs)
    out_full = np.empty((2, C, H, W), np.float32)
    for core in range(8):
        s, q = divmod(core, 4)
        out_full[s, :, q * QROWS:(q + 1) * QROWS, :] = \
            res.results[core]["out"].reshape(C, QROWS, W)
    if run_kwargs:
        return out_full, res
    return out_full


# revision 10
# speedup vs baseline: 443.0244x; 443.0244x over previous
"""Trainium2 Bass kernel for the LDE guided-attention module.

Sharding: 8 cores = 2 samples x 4 row-quarters of the N=9216 attention rows.
Zero cross-core communication: each core redundantly computes the (cheap)
conv trunk for its sample, then its quarter of the softmax(d1@d2)@c1 rows
flash-attention style -- the [N,N] map never leaves PSUM/SBUF.

Layouts (per core, sample s=core//4, quarter q=core%4):
  - trunk conv3x3 as 9 offset-matmuls over a zero-padded [64, 98, 98] slab
  - d2   [32, 9216] channel-major (lhsT tiles for scores)
  - c1aug [128, 72, 33] = c1 in N-major layout + ones column (fused rowsum)
  - d1q  [32, 2304], d0q [64, 2304] from a 26-row halo slab of depth
  - scores S^T tile [128, Rb] = matmul(lhsT=d2_tile, rhs=d1q_blk); exp on ACT;
    guided^T+rowsum accumulate via matmul(lhsT=c1aug_tile, rhs=expS)
  - epilogue: out = (wch4 @ guided^T) * (1/rowsum) + d0q
All matmul operands bitcast to float32r: full fp32 data at 1 cycle/row.
"""

import sys

for _p in ("/opt/trn_rl_repo",):
    if _p not in sys.path:
        sys.path.insert(0, _p)

import numpy as np

import concourse.bass as bass
import concourse.bacc as bacc
import concourse.mybir as mybir
from concourse import tile
from concourse.bass_utils import run_bass_kernel_spmd

F32 = mybir.dt.float32
F32R = mybir.dt.float32r
AF = mybir.ActivationFunctionType

C = 64          # channels
CQ = 32         # C // 2
H = W = 96
N = H * W       # 9216
NT = N // 128   # 72 column tiles
QROWS = 24      # image rows per quarter
NQ = QROWS * W  # 2304 attention rows per core
PW = 98         # padded width
CHUNK_ROWS = 4
CHUNK = CHUNK_ROWS * W  # 384
BLOCKS = [(0, 512), (512, 512), (1024, 512), (1536, 512), (2048, 256)]

_cache = {}


def _r(ap):
    return ap


def _trunk_chunk(nc, tc, kpool, ps, slab, row0, w1t_sb, w2t_sb, b1_sb, b2_sb,
                 a1, a2, out_ap=None):
    """conv3x3+PReLU then conv1x1+PReLU for 4 image rows starting at
    slab row row0 (slab has 1 halo row on top). Returns [64, 384] AP."""
    psc = ps.tile([C, CHUNK], F32, tag="pscv")
    for k in range(9):
        ky, kx = divmod(k, 3)
        rhs = slab[:, row0 + ky: row0 + ky + CHUNK_ROWS, kx: kx + W]
        nc.tensor.matmul(psc[:], _r(w1t_sb[:, k * C:(k + 1) * C]), _r(rhs),
                         start=(k == 0), stop=(k == 8))
    # prelu(y+b, a) = a*(y+b) + (1-a)*relu(y+b); relu(s*y+s*b) = s*relu(y+b)
    r = kpool.tile([C, CHUNK], F32, tag="tr")
    t = kpool.tile([C, CHUNK], F32, tag="tt")
    nc.scalar.activation(r[:], psc[:], AF.Relu, bias=b1_sb[:, 1:2], scale=1.0 - a1)
    nc.scalar.activation(t[:], psc[:], AF.Identity, bias=b1_sb[:, 0:1], scale=a1)
    pre = kpool.tile([C, CHUNK], F32, tag="tp")
    nc.vector.tensor_add(pre[:], r[:], t[:])

    psc2 = ps.tile([C, CHUNK], F32, tag="pscv")
    nc.tensor.matmul(psc2[:], _r(w2t_sb[:]), _r(pre[:]), start=True, stop=True)
    r2 = kpool.tile([C, CHUNK], F32, tag="tr")
    t2 = kpool.tile([C, CHUNK], F32, tag="tt")
    nc.scalar.activation(r2[:], psc2[:], AF.Relu, bias=b2_sb[:, 1:2], scale=1.0 - a2)
    nc.scalar.activation(t2[:], psc2[:], AF.Identity, bias=b2_sb[:, 0:1], scale=a2)
    if out_ap is None:
        c = kpool.tile([C, CHUNK], F32, tag="tc")
        out_ap = c[:]
    nc.vector.tensor_add(out_ap, r2[:], t2[:])
    return out_ap


def _build(a1: float, a2: float, loop_n: int = 1):
    nc = bacc.Bacc(None, target_bir_lowering=False)
    xr = nc.declare_dram_parameter("xr", [C, N], F32, isOutput=False)
    xd = nc.declare_dram_parameter("xd", [C, N], F32, isOutput=False)
    xdq = nc.declare_dram_parameter("xdq", [C, 26 * W], F32, isOutput=False)
    w1t = nc.declare_dram_parameter("w1t", [C, 9 * C], F32, isOutput=False)
    w2t = nc.declare_dram_parameter("w2t", [C, C], F32, isOutput=False)
    wch1t = nc.declare_dram_parameter("wch1t", [C, CQ], F32, isOutput=False)
    wch2t = nc.declare_dram_parameter("wch2t", [C, CQ], F32, isOutput=False)
    wch3t = nc.declare_dram_parameter("wch3t", [C, CQ], F32, isOutput=False)
    wch4t = nc.declare_dram_parameter("wch4t", [CQ, C], F32, isOutput=False)
    b1 = nc.declare_dram_parameter("b1", [C, 2], F32, isOutput=False)
    b2 = nc.declare_dram_parameter("b2", [C, 2], F32, isOutput=False)
    out = nc.declare_dram_parameter("out", [C, NQ], F32, isOutput=True)

    with tile.TileContext(nc) as tc:
        with (
            tc.tile_pool(name="const", bufs=1) as cpool,
            tc.tile_pool(name="xpad", bufs=1) as xpool,
            tc.tile_pool(name="big", bufs=1) as bpool,
            tc.tile_pool(name="chunk", bufs=3) as kpool,
            tc.tile_pool(name="pt", bufs=3) as ptpool,
            tc.tile_pool(name="ep", bufs=2) as eppool,
            tc.tile_pool(name="ps_s", bufs=2, space="PSUM") as ps_s,
            tc.tile_pool(name="ps_g", bufs=2, space="PSUM") as ps_g,
            tc.tile_pool(name="ps_m", bufs=3, space="PSUM") as ps_m,
        ):
            # ---- constants ----
            w1t_sb = cpool.tile([C, 9 * C], F32)
            nc.sync.dma_start(w1t_sb[:], w1t[:])
            w2t_sb = cpool.tile([C, C], F32)
            nc.sync.dma_start(w2t_sb[:], w2t[:])
            wch1t_sb = cpool.tile([C, CQ], F32)
            nc.sync.dma_start(wch1t_sb[:], wch1t[:])
            wch2t_sb = cpool.tile([C, CQ], F32)
            nc.sync.dma_start(wch2t_sb[:], wch2t[:])
            wch3t_sb = cpool.tile([C, CQ], F32)
            nc.sync.dma_start(wch3t_sb[:], wch3t[:])
            wch4t_sb = cpool.tile([CQ, C], F32)
            nc.sync.dma_start(wch4t_sb[:], wch4t[:])
            b1_sb = cpool.tile([C, 2], F32)
            nc.sync.dma_start(b1_sb[:], b1[:])
            b2_sb = cpool.tile([C, 2], F32)
            nc.sync.dma_start(b2_sb[:], b2[:])
            ones_sb = cpool.tile([1, C], F32)
            nc.vector.memset(ones_sb[:], 1.0)

            import contextlib
            loop_cm = tc.For_i(0, loop_n, 1) if loop_n > 1 else \
                contextlib.nullcontext()
            with loop_cm:
                _body(nc, tc, locals())

    nc.finalize()
    return nc


def _body(nc, tc, env):
    (cpool, xpool, bpool, kpool, ptpool, eppool, ps_s, ps_g, ps_m) = (
        env[k] for k in ("cpool", "xpool", "bpool", "kpool", "ptpool",
                         "eppool", "ps_s", "ps_g", "ps_m"))
    (w1t_sb, w2t_sb, wch1t_sb, wch2t_sb, wch3t_sb, wch4t_sb, b1_sb, b2_sb,
     ones_sb) = (env[k] for k in ("w1t_sb", "w2t_sb", "wch1t_sb", "wch2t_sb",
                                  "wch3t_sb", "wch4t_sb", "b1_sb", "b2_sb",
                                  "ones_sb"))
    (xr, xd, xdq, out, a1, a2) = (env[k] for k in
                                  ("xr", "xd", "xdq", "out", "a1", "a2"))
    if True:
        if True:

            # ---- persistent intermediates ----
            d2_sb = bpool.tile([CQ, N], F32)           # scores lhsT source
            c1aug = bpool.tile([128, NT, CQ + 1], F32)  # c1 N-major + ones col
            d1q = bpool.tile([CQ, NQ], F32)
            d0q = bpool.tile([C, NQ], F32)
            nc.vector.memset(c1aug[:, :, CQ:CQ + 1], 1.0)

            # ---- depth quarter (halo slab): d0q, d1q ----
            dq_slab = xpool.tile([C, 26, PW], F32, tag="dqslab")
            nc.vector.memset(dq_slab[:, :, 0:1], 0.0)
            nc.vector.memset(dq_slab[:, :, PW - 1:PW], 0.0)
            nc.sync.dma_start(
                dq_slab[:, :, 1:W + 1],
                xdq[:].rearrange("c (r w) -> c r w", w=W),
            )
            for j in range(NQ // CHUNK):
                sl = slice(j * CHUNK, (j + 1) * CHUNK)
                _trunk_chunk(nc, tc, kpool, ps_m, dq_slab, 4 * j, w1t_sb,
                             w2t_sb, b1_sb, b2_sb, a1, a2, out_ap=d0q[:, sl])
                psq = ps_m.tile([CQ, CHUNK], F32, tag="pscv")
                nc.tensor.matmul(psq[:], _r(wch2t_sb[:]), _r(d0q[:, sl]),
                                 start=True, stop=True)
                nc.vector.tensor_copy(d1q[:, sl], psq[:])

            # ---- rgb trunk -> c1aug (N-major) ----
            slab = xpool.tile([C, PW, PW], F32, tag="slab")
            for edge in (0, PW - 1):
                nc.vector.memset(slab[:, edge, :], 0.0)
                nc.vector.memset(slab[:, 1:PW - 1, edge:edge + 1], 0.0)
            xr_r = xr[:].rearrange("c (h w) -> c h w", w=W)
            for piece in range(4):
                rs = slice(piece * QROWS, (piece + 1) * QROWS)
                nc.sync.dma_start(slab[:, 1 + piece * QROWS:1 + (piece + 1) * QROWS, 1:W + 1],
                                  xr_r[:, rs, :])
            for j in range(N // CHUNK):
                c = _trunk_chunk(nc, tc, kpool, ps_m, slab, 4 * j, w1t_sb,
                                 w2t_sb, b1_sb, b2_sb, a1, a2)
                for i in range(3):
                    ti = 3 * j + i
                    psn = ps_m.tile([128, CQ], F32, tag="pscv")
                    nc.tensor.matmul(psn[:], _r(c[:, i * 128:(i + 1) * 128]),
                                     _r(wch1t_sb[:]), start=True, stop=True)
                    nc.vector.tensor_copy(c1aug[:, ti, 0:CQ], psn[:])

            # ---- depth trunk -> d2 (channel-major) ----
            slab2 = xpool.tile([C, PW, PW], F32, tag="slab")
            for edge in (0, PW - 1):
                nc.vector.memset(slab2[:, edge, :], 0.0)
                nc.vector.memset(slab2[:, 1:PW - 1, edge:edge + 1], 0.0)
            xd_r = xd[:].rearrange("c (h w) -> c h w", w=W)
            for piece in range(4):
                rs = slice(piece * QROWS, (piece + 1) * QROWS)
                nc.sync.dma_start(slab2[:, 1 + piece * QROWS:1 + (piece + 1) * QROWS, 1:W + 1],
                                  xd_r[:, rs, :])
            for j in range(N // CHUNK):
                d = _trunk_chunk(nc, tc, kpool, ps_m, slab2, 4 * j, w1t_sb,
                                 w2t_sb, b1_sb, b2_sb, a1, a2)
                psd = ps_m.tile([CQ, CHUNK], F32, tag="pscv")
                nc.tensor.matmul(psd[:], _r(wch3t_sb[:]), _r(d),
                                 start=True, stop=True)
                nc.vector.tensor_copy(d2_sb[:, j * CHUNK:(j + 1) * CHUNK], psd[:])

            # ---- streaming attention over row blocks ----
            for (o, rb) in BLOCKS:
                ps_acc = ps_g.tile([CQ + 1, rb], F32, tag="psg")
                for t in range(NT):
                    ps_sc = ps_s.tile([128, rb], F32, tag="pss")
                    nc.tensor.matmul(ps_sc[:], _r(d2_sb[:, t * 128:(t + 1) * 128]),
                                     _r(d1q[:, o:o + rb]), start=True, stop=True)
                    pT = ptpool.tile([128, rb], F32, tag="pt")
                    nc.scalar.activation(pT[:], ps_sc[:], AF.Exp)
                    nc.tensor.matmul(ps_acc[:], _r(c1aug[:, t, :]), _r(pT[:]),
                                     start=(t == 0), stop=(t == NT - 1),
                                     skip_group_check=True)
                g_sb = eppool.tile([CQ, rb], F32, tag="gsb")
                nc.vector.tensor_copy(g_sb[:], ps_acc[0:CQ, :])
                sum_sb = eppool.tile([1, rb], F32, tag="ssb")
                nc.vector.tensor_copy(sum_sb[:], ps_acc[CQ:CQ + 1, :])
                ps_b = ps_m.tile([C, rb], F32, tag="pscv")
                nc.tensor.matmul(ps_b[:], _r(ones_sb[:]), _r(sum_sb[:]),
                                 start=True, stop=True)
                rcp = eppool.tile([C, rb], F32, tag="rcp")
                nc.vector.reciprocal(rcp[:], ps_b[:])
                ps_o = ps_m.tile([C, rb], F32, tag="pscv")
                nc.tensor.matmul(ps_o[:], _r(wch4t_sb[:]), _r(g_sb[:]),
                                 start=True, stop=True)
                o1 = eppool.tile([C, rb], F32, tag="o1")
                nc.vector.tensor_mul(o1[:], ps_o[:], rcp[:])
                osb = eppool.tile([C, rb], F32, tag="osb")
                nc.vector.tensor_add(osb[:], o1[:], d0q[:, o:o + rb])
                nc.sync.dma_start(out[:, o:o + rb], osb[:])


def _prep_inputs(rgb, depth, w1, b1, a1, w2, b2, a2, wch1, wch2, wch3, wch4):
    rgb = np.asarray(rgb, np.float32)
    depth = np.asarray(depth, np.float32)
    # w1t[ci, (ky*3+kx)*C + co]
    w1t = np.ascontiguousarray(
        np.transpose(np.asarray(w1, np.float32), (1, 2, 3, 0)).reshape(C, 9 * C))
    w2t = np.ascontiguousarray(np.asarray(w2, np.float32)[:, :, 0, 0].T)
    wch1t = np.ascontiguousarray(np.asarray(wch1, np.float32)[:, :, 0, 0].T)
    wch2t = np.ascontiguousarray(np.asarray(wch2, np.float32)[:, :, 0, 0].T)
    wch3t = np.ascontiguousarray(np.asarray(wch3, np.float32)[:, :, 0, 0].T)
    wch4t = np.ascontiguousarray(np.asarray(wch4, np.float32)[:, :, 0, 0].T)
    a1f = float(np.asarray(a1)); a2f = float(np.asarray(a2))
    b1a = np.stack([a1f * np.asarray(b1, np.float32),
                    (1.0 - a1f) * np.asarray(b1, np.float32)], axis=1)
    b2a = np.stack([a2f * np.asarray(b2, np.float32),
                    (1.0 - a2f) * np.asarray(b2, np.float32)], axis=1)

    in_maps = []
    for core in range(8):
        s, q = divmod(core, 4)
        xdq = np.zeros((C, 26, W), np.float32)
        for r_slab in range(26):
            r_img = q * QROWS - 1 + r_slab
            if 0 <= r_img < H:
                xdq[:, r_slab, :] = depth[s, :, r_img, :]
        in_maps.append({
            "xr": np.ascontiguousarray(rgb[s].reshape(C, N)),
            "xd": np.ascontiguousarray(depth[s].reshape(C, N)),
            "xdq": np.ascontiguousarray(xdq.reshape(C, 26 * W)),
            "w1t": w1t, "w2t": w2t,
            "wch1t": wch1t, "wch2t": wch2t, "wch3t": wch3t, "wch4t": wch4t,
            "b1": np.ascontiguousarray(b1a), "b2": np.ascontiguousarray(b2a),
        })
    return in_maps, (a1f, a2f)


def kernel(rgb, depth, w1, b1, a1, w2, b2, a2, wch1, wch2, wch3, wch4,
           _loop_n=1, **run_kwargs):
    in_maps, (a1f, a2f) = _prep_inputs(rgb, depth, w1, b1, a1, w2, b2, a2,
                                       wch1, wch2, wch3, wch4)
    key = (a1f, a2f, _loop_n)
    if key not in _cache:
        _cache[key] = _build(a1f, a2f, loop_n=_loop_n)
    nc = _cache[key]
    res = run_bass_kernel_spmd(nc, in_maps, list(range(8)), **run_kwargs)
    out_full = np.empty((2, C, H, W), np.float32)
    for core in range(8):
        s, q = divmod(core, 4)
        out_full[s, :, q * QROWS:(q + 1) * QROWS, :] = \
            res.results[core]["out"].reshape(C, QROWS, W)
    if run_kwargs:
        return out_full, res
    return out_full


# revision 12
# speedup vs baseline: 1897.2521x; 4.2825x over previous
"""Trainium2 Bass kernel for the LDE guided-attention module.

Sharding: 8 cores = 2 samples x 4 row-quarters of the N=9216 attention rows.
Zero cross-core communication: each core redundantly computes the (cheap)
conv trunk for its sample, then its quarter of the softmax(d1@d2)@c1 rows
flash-attention style -- the [N,N] map never leaves PSUM/SBUF.

Layouts (per core, sample s=core//4, quarter q=core%4):
  - trunk conv3x3 as 9 offset-matmuls over a zero-padded [64, 98, 98] slab
  - d2   [32, 9216] channel-major (lhsT tiles for scores)
  - c1aug [128, 72, 33] = c1 in N-major layout + ones column (fused rowsum)
  - d1q  [32, 2304], d0q [64, 2304] from a 26-row halo slab of depth
  - scores S^T tile [128, Rb] = matmul(lhsT=d2_tile, rhs=d1q_blk); exp on ACT;
    guided^T+rowsum accumulate via matmul(lhsT=c1aug_tile, rhs=expS)
  - epilogue: out = (wch4 @ guided^T) * (1/rowsum) + d0q
All matmul operands bitcast to float32r: full fp32 data at 1 cycle/row.
"""

import sys

for _p in ("/opt/trn_rl_repo",):
    if _p not in sys.path:
        sys.path.insert(0, _p)

import numpy as np

import concourse.bass as bass
import concourse.bacc as bacc
import concourse.mybir as mybir
from concourse import tile
from concourse.bass_utils import run_bass_kernel_spmd

F32 = mybir.dt.float32
F32R = mybir.dt.float32r
AF = mybir.ActivationFunctionType

C = 64          # channels
CQ = 32         # C // 2
H = W = 96
N = H * W       # 9216
NT = N // 128   # 72 column tiles
QROWS = 24      # image rows per quarter
NQ = QROWS * W  # 2304 attention rows per core
PW = 98         # padded width
CHUNK_ROWS = 4
CHUNK = CHUNK_ROWS * W  # 384
BLOCKS = [(0, 512), (512, 512), (1024, 512), (1536, 512), (2048, 256)]

_cache = {}


def _r(ap):
    return ap


def _trunk_chunk(nc, tc, kpool, ps, slab, row0, w1t_sb, w2t_sb, b1_sb, b2_sb,
                 a1, a2, out_ap=None):
    """conv3x3+PReLU then conv1x1+PReLU for 4 image rows starting at
    slab row row0 (slab has 1 halo row on top). Returns [64, 384] AP."""
    psc = ps.tile([C, CHUNK], F32, tag="pscv")
    for k in range(9):
        ky, kx = divmod(k, 3)
        rhs = slab[:, row0 + ky: row0 + ky + CHUNK_ROWS, kx: kx + W]
        nc.tensor.matmul(psc[:], _r(w1t_sb[:, k * C:(k + 1) * C]), _r(rhs),
                         start=(k == 0), stop=(k == 8))
    # prelu(y+b, a) = a*(y+b) + (1-a)*relu(y+b); relu(s*y+s*b) = s*relu(y+b)
    r = kpool.tile([C, CHUNK], F32, tag="tr")
    t = kpool.tile([C, CHUNK], F32, tag="tt")
    nc.scalar.activation(r[:], psc[:], AF.Relu, bias=b1_sb[:, 1:2], scale=1.0 - a1)
    nc.scalar.activation(t[:], psc[:], AF.Identity, bias=b1_sb[:, 0:1], scale=a1)
    pre = kpool.tile([C, CHUNK], F32R, tag="tp")
    nc.vector.tensor_add(pre[:], r[:], t[:])

    psc2 = ps.tile([C, CHUNK], F32, tag="pscv")
    nc.tensor.matmul(psc2[:], _r(w2t_sb[:]), _r(pre[:]), start=True, stop=True)
    r2 = kpool.tile([C, CHUNK], F32, tag="tr")
    t2 = kpool.tile([C, CHUNK], F32, tag="tt")
    nc.scalar.activation(r2[:], psc2[:], AF.Relu, bias=b2_sb[:, 1:2], scale=1.0 - a2)
    nc.scalar.activation(t2[:], psc2[:], AF.Identity, bias=b2_sb[:, 0:1], scale=a2)
    if out_ap is None:
        c = kpool.tile([C, CHUNK], F32R, tag="tc")
        out_ap = c[:]
    nc.vector.tensor_add(out_ap, r2[:], t2[:])
    return out_ap


def _build(a1: float, a2: float, loop_n: int = 1):
    nc = bacc.Bacc(None, target_bir_lowering=False)
    xr = nc.declare_dram_parameter("xr", [C, N], F32R, isOutput=False)
    xd = nc.declare_dram_parameter("xd", [C, N], F32R, isOutput=False)
    xdq = nc.declare_dram_parameter("xdq", [C, 26 * W], F32R, isOutput=False)
    w1t = nc.declare_dram_parameter("w1t", [C, 9 * C], F32R, isOutput=False)
    w2t = nc.declare_dram_parameter("w2t", [C, C], F32R, isOutput=False)
    wch1t = nc.declare_dram_parameter("wch1t", [C, CQ], F32R, isOutput=False)
    wch2t = nc.declare_dram_parameter("wch2t", [C, CQ], F32R, isOutput=False)
    wch3t = nc.declare_dram_parameter("wch3t", [C, CQ], F32R, isOutput=False)
    wch4t = nc.declare_dram_parameter("wch4t", [CQ, C], F32R, isOutput=False)
    b1 = nc.declare_dram_parameter("b1", [C, 2], F32, isOutput=False)
    b2 = nc.declare_dram_parameter("b2", [C, 2], F32, isOutput=False)
    zz = nc.declare_dram_parameter("zz", [C, PW], F32R, isOutput=False)
    kones = nc.declare_dram_parameter("kones", [128, NT], F32R, isOutput=False)
    out = nc.declare_dram_parameter("out", [C, NQ], F32, isOutput=True)

    with tile.TileContext(nc) as tc:
        with (
            tc.tile_pool(name="const", bufs=1) as cpool,
            tc.tile_pool(name="xpad", bufs=1) as xpool,
            tc.tile_pool(name="big", bufs=1) as bpool,
            tc.tile_pool(name="chunk", bufs=3) as kpool,
            tc.tile_pool(name="pt", bufs=3) as ptpool,
            tc.tile_pool(name="ep", bufs=2) as eppool,
            tc.tile_pool(name="ps_s", bufs=2, space="PSUM") as ps_s,
            tc.tile_pool(name="ps_g", bufs=2, space="PSUM") as ps_g,
            tc.tile_pool(name="ps_m", bufs=3, space="PSUM") as ps_m,
        ):
            # ---- constants ----
            w1t_sb = cpool.tile([C, 9 * C], F32R)
            nc.sync.dma_start(w1t_sb[:], w1t[:])
            w2t_sb = cpool.tile([C, C], F32R)
            nc.sync.dma_start(w2t_sb[:], w2t[:])
            wch1t_sb = cpool.tile([C, CQ], F32R)
            nc.sync.dma_start(wch1t_sb[:], wch1t[:])
            wch2t_sb = cpool.tile([C, CQ], F32R)
            nc.sync.dma_start(wch2t_sb[:], wch2t[:])
            wch3t_sb = cpool.tile([C, CQ], F32R)
            nc.sync.dma_start(wch3t_sb[:], wch3t[:])
            wch4t_sb = cpool.tile([CQ, C], F32R)
            nc.sync.dma_start(wch4t_sb[:], wch4t[:])
            b1_sb = cpool.tile([C, 2], F32)
            nc.sync.dma_start(b1_sb[:], b1[:])
            b2_sb = cpool.tile([C, 2], F32)
            nc.sync.dma_start(b2_sb[:], b2[:])
            ones_sb = cpool.tile([1, C], F32R)
            nc.sync.dma_start(ones_sb[:], kones[0:1, 0:C])

            import contextlib
            loop_cm = tc.For_i(0, loop_n, 1) if loop_n > 1 else \
                contextlib.nullcontext()
            with loop_cm:
                _body(nc, tc, locals())

    nc.finalize()
    return nc


def _body(nc, tc, env):
    (cpool, xpool, bpool, kpool, ptpool, eppool, ps_s, ps_g, ps_m) = (
        env[k] for k in ("cpool", "xpool", "bpool", "kpool", "ptpool",
                         "eppool", "ps_s", "ps_g", "ps_m"))
    (w1t_sb, w2t_sb, wch1t_sb, wch2t_sb, wch3t_sb, wch4t_sb, b1_sb, b2_sb,
     ones_sb) = (env[k] for k in ("w1t_sb", "w2t_sb", "wch1t_sb", "wch2t_sb",
                                  "wch3t_sb", "wch4t_sb", "b1_sb", "b2_sb",
                                  "ones_sb"))
    (xr, xd, xdq, out, a1, a2, zz, kones) = (env[k] for k in
                                  ("xr", "xd", "xdq", "out", "a1", "a2",
                                   "zz", "kones"))
    if True:
        if True:

            # ---- persistent intermediates ----
            d2_sb = bpool.tile([CQ, N], F32R)           # scores lhsT source
            c1aug = bpool.tile([128, NT, CQ + 1], F32R)  # c1 N-major + ones col
            d1q = bpool.tile([CQ, NQ], F32R)
            d0q = bpool.tile([C, NQ], F32R)
            nc.sync.dma_start(c1aug[:, :, CQ:CQ + 1], kones[:].unsqueeze(2))

            # ---- depth quarter (halo slab): d0q, d1q ----
            dq_slab = xpool.tile([C, 26, PW], F32R, tag="dqslab")
            nc.sync.dma_start(dq_slab[:, :, 0:1], zz[:, 0:26].unsqueeze(2))
            nc.sync.dma_start(dq_slab[:, :, PW - 1:PW], zz[:, 0:26].unsqueeze(2))
            nc.sync.dma_start(
                dq_slab[:, :, 1:W + 1],
                xdq[:].rearrange("c (r w) -> c r w", w=W),
            )
            for j in range(NQ // CHUNK):
                sl = slice(j * CHUNK, (j + 1) * CHUNK)
                _trunk_chunk(nc, tc, kpool, ps_m, dq_slab, 4 * j, w1t_sb,
                             w2t_sb, b1_sb, b2_sb, a1, a2, out_ap=d0q[:, sl])
                psq = ps_m.tile([CQ, CHUNK], F32, tag="pscv")
                nc.tensor.matmul(psq[:], _r(wch2t_sb[:]), _r(d0q[:, sl]),
                                 start=True, stop=True)
                nc.vector.tensor_copy(d1q[:, sl], psq[:])

            # ---- rgb trunk -> c1aug (N-major) ----
            slab = xpool.tile([C, PW, PW], F32R, tag="slab")
            for edge in (0, PW - 1):
                nc.sync.dma_start(slab[:, edge, :], zz[:])
                nc.sync.dma_start(slab[:, 1:PW - 1, edge:edge + 1],
                                  zz[:, 0:PW - 2].unsqueeze(2))
            xr_r = xr[:].rearrange("c (h w) -> c h w", w=W)
            for piece in range(4):
                rs = slice(piece * QROWS, (piece + 1) * QROWS)
                nc.sync.dma_start(slab[:, 1 + piece * QROWS:1 + (piece + 1) * QROWS, 1:W + 1],
                                  xr_r[:, rs, :])
            for j in range(N // CHUNK):
                c = _trunk_chunk(nc, tc, kpool, ps_m, slab, 4 * j, w1t_sb,
                                 w2t_sb, b1_sb, b2_sb, a1, a2)
                for i in range(3):
                    ti = 3 * j + i
                    psn = ps_m.tile([128, CQ], F32, tag="pscv")
                    nc.tensor.matmul(psn[:], _r(c[:, i * 128:(i + 1) * 128]),
                                     _r(wch1t_sb[:]), start=True, stop=True)
                    nc.vector.tensor_copy(c1aug[:, ti, 0:CQ], psn[:])

            # ---- depth trunk -> d2 (channel-major) ----
            slab2 = xpool.tile([C, PW, PW], F32R, tag="slab")
            for edge in (0, PW - 1):
                nc.sync.dma_start(slab2[:, edge, :], zz[:])
                nc.sync.dma_start(slab2[:, 1:PW - 1, edge:edge + 1],
                                  zz[:, 0:PW - 2].unsqueeze(2))
            xd_r = xd[:].rearrange("c (h w) -> c h w", w=W)
            for piece in range(4):
                rs = slice(piece * QROWS, (piece + 1) * QROWS)
                nc.sync.dma_start(slab2[:, 1 + piece * QROWS:1 + (piece + 1) * QROWS, 1:W + 1],
                                  xd_r[:, rs, :])
            for j in range(N // CHUNK):
                d = _trunk_chunk(nc, tc, kpool, ps_m, slab2, 4 * j, w1t_sb,
                                 w2t_sb, b1_sb, b2_sb, a1, a2)
                psd = ps_m.tile([CQ, CHUNK], F32, tag="pscv")
                nc.tensor.matmul(psd[:], _r(wch3t_sb[:]), _r(d),
                                 start=True, stop=True)
                nc.vector.tensor_copy(d2_sb[:, j * CHUNK:(j + 1) * CHUNK], psd[:])

            # ---- streaming attention over row blocks ----
            for (o, rb) in BLOCKS:
                ps_acc = ps_g.tile([CQ + 1, rb], F32, tag="psg")
                for t in range(NT):
                    ps_sc = ps_s.tile([128, rb], F32, tag="pss")
                    nc.tensor.matmul(ps_sc[:], _r(d2_sb[:, t * 128:(t + 1) * 128]),
                                     _r(d1q[:, o:o + rb]), start=True, stop=True)
                    pT = ptpool.tile([128, rb], F32R, tag="pt")
                    nc.scalar.activation(pT[:], ps_sc[:], AF.Exp)
                    nc.tensor.matmul(ps_acc[:], _r(c1aug[:, t, :]), _r(pT[:]),
                                     start=(t == 0), stop=(t == NT - 1),
                                     skip_group_check=True)
                g_sb = eppool.tile([CQ, rb], F32R, tag="gsb")
                nc.vector.tensor_copy(g_sb[:], ps_acc[0:CQ, :])
                sum_sb = eppool.tile([1, rb], F32R, tag="ssb")
                nc.vector.tensor_copy(sum_sb[:], ps_acc[CQ:CQ + 1, :])
                ps_b = ps_m.tile([C, rb], F32, tag="pscv")
                nc.tensor.matmul(ps_b[:], _r(ones_sb[:]), _r(sum_sb[:]),
                                 start=True, stop=True)
                rcp = eppool.tile([C, rb], F32, tag="rcp")
                nc.vector.reciprocal(rcp[:], ps_b[:])
                ps_o = ps_m.tile([C, rb], F32, tag="pscv")
                nc.tensor.matmul(ps_o[:], _r(wch4t_sb[:]), _r(g_sb[:]),
                                 start=True, stop=True)
                o1 = eppool.tile([C, rb], F32, tag="o1")
                nc.vector.tensor_mul(o1[:], ps_o[:], rcp[:])
                osb = eppool.tile([C, rb], F32, tag="osb")
                nc.vector.tensor_add(osb[:], o1[:], d0q[:, o:o + rb].bitcast(F32))
                nc.sync.dma_start(out[:, o:o + rb], osb[:])


def _prep_inputs(rgb, depth, w1, b1, a1, w2, b2, a2, wch1, wch2, wch3, wch4):
    rgb = np.asarray(rgb, np.float32)
    depth = np.asarray(depth, np.float32)
    # w1t[ci, (ky*3+kx)*C + co]
    w1t = np.ascontiguousarray(
        np.transpose(np.asarray(w1, np.float32), (1, 2, 3, 0)).reshape(C, 9 * C))
    w2t = np.ascontiguousarray(np.asarray(w2, np.float32)[:, :, 0, 0].T)
    wch1t = np.ascontiguousarray(np.asarray(wch1, np.float32)[:, :, 0, 0].T)
    wch2t = np.ascontiguousarray(np.asarray(wch2, np.float32)[:, :, 0, 0].T)
    wch3t = np.ascontiguousarray(np.asarray(wch3, np.float32)[:, :, 0, 0].T)
    wch4t = np.ascontiguousarray(np.asarray(wch4, np.float32)[:, :, 0, 0].T)
    a1f = float(np.asarray(a1)); a2f = float(np.asarray(a2))
    b1a = np.stack([a1f * np.asarray(b1, np.float32),
                    (1.0 - a1f) * np.asarray(b1, np.float32)], axis=1)
    b2a = np.stack([a2f * np.asarray(b2, np.float32),
                    (1.0 - a2f) * np.asarray(b2, np.float32)], axis=1)

    in_maps = []
    for core in range(8):
        s, q = divmod(core, 4)
        xdq = np.zeros((C, 26, W), np.float32)
        for r_slab in range(26):
            r_img = q * QROWS - 1 + r_slab
            if 0 <= r_img < H:
                xdq[:, r_slab, :] = depth[s, :, r_img, :]
        in_maps.append({
            "xr": np.ascontiguousarray(rgb[s].reshape(C, N)),
            "xd": np.ascontiguousarray(depth[s].reshape(C, N)),
            "xdq": np.ascontiguousarray(xdq.reshape(C, 26 * W)),
            "w1t": w1t, "w2t": w2t,
            "wch1t": wch1t, "wch2t": wch2t, "wch3t": wch3t, "wch4t": wch4t,
            "b1": np.ascontiguousarray(b1a), "b2": np.ascontiguousarray(b2a),
            "zz": np.zeros((C, PW), np.float32),
            "kones": np.ones((128, NT), np.float32),
        })
    return in_maps, (a1f, a2f)


def kernel(rgb, depth, w1, b1, a1, w2, b2, a2, wch1, wch2, wch3, wch4,
           _loop_n=1, **run_kwargs):
    in_maps, (a1f, a2f) = _prep_inputs(rgb, depth, w1, b1, a1, w2, b2, a2,
                                       wch1, wch2, wch3, wch4)
    key = (a1f, a2f, _loop_n)
    if key not in _cache:
        _cache[key] = _build(a1f, a2f, loop_n=_loop_n)
    nc = _cache[key]
    res = run_bass_kernel_spmd(nc, in_maps, list(range(8)), **run_kwargs)
    out_full = np.empty((2, C, H, W), np.float32)
    for core in range(8):
        s, q = divmod(core, 4)
        out_full[s, :, q * QROWS:(q + 1) * QROWS, :] = \
            res.results[core]["out"].reshape(C, QROWS, W)
    if run_kwargs:
        return out_full, res
    return out_full


# revision 13
# speedup vs baseline: 2485.0386x; 1.3098x over previous
"""Trainium2 Bass kernel for the LDE guided-attention module.

Sharding: 8 cores = 2 samples x 4 row-quarters of the N=9216 attention rows.
Zero cross-core communication: each core redundantly computes the (cheap)
conv trunk for its sample, then its quarter of the softmax(d1@d2)@c1 rows
flash-attention style -- the [N,N] map never leaves PSUM/SBUF.

Layouts (per core, sample s=core//4, quarter q=core%4):
  - trunk conv3x3 as 9 offset-matmuls over a zero-padded [64, 98, 98] slab
  - d2   [32, 9216] channel-major (lhsT tiles for scores)
  - c1aug [128, 72, 33] = c1 in N-major layout + ones column (fused rowsum)
  - d1q  [32, 2304], d0q [64, 2304] from a 26-row halo slab of depth
  - scores S^T tile [128, Rb] = matmul(lhsT=d2_tile, rhs=d1q_blk); exp on ACT;
    guided^T+rowsum accumulate via matmul(lhsT=c1aug_tile, rhs=expS)
  - epilogue: out = (wch4 @ guided^T) * (1/rowsum) + d0q
All matmul operands bitcast to float32r: full fp32 data at 1 cycle/row.
"""

import sys

for _p in ("/opt/trn_rl_repo",):
    if _p not in sys.path:
        sys.path.insert(0, _p)

import numpy as np

import concourse.bass as bass
import concourse.bacc as bacc
import concourse.mybir as mybir
from concourse import tile
from concourse.bass_utils import run_bass_kernel_spmd

F32 = mybir.dt.float32
F32R = mybir.dt.float32r
AF = mybir.ActivationFunctionType

C = 64          # channels
CQ = 32         # C // 2
H = W = 96
N = H * W       # 9216
NT = N // 128   # 72 column tiles
QROWS = 24      # image rows per quarter
NQ = QROWS * W  # 2304 attention rows per core
PW = 98         # padded width
CHUNK_ROWS = 4
CHUNK = CHUNK_ROWS * W  # 384
BLOCKS = [(0, 512), (512, 512), (1024, 512), (1536, 512), (2048, 256)]

_cache = {}


def _r(ap):
    return ap


def _trunk_chunk(nc, tc, kpool, ps, slab, row0, w1t_sb, w2t_sb, b1_sb, b2_sb,
                 a1, a2, out_ap=None):
    """conv3x3+PReLU then conv1x1+PReLU for 4 image rows starting at
    slab row row0 (slab has 1 halo row on top). Returns [64, 384] AP."""
    psc = ps.tile([C, CHUNK], F32, tag="pscv")
    for k in range(9):
        ky, kx = divmod(k, 3)
        rhs = slab[:, row0 + ky: row0 + ky + CHUNK_ROWS, kx: kx + W]
        nc.tensor.matmul(psc[:], _r(w1t_sb[:, k * C:(k + 1) * C]), _r(rhs),
                         start=(k == 0), stop=(k == 8))
    pre = kpool.tile([C, CHUNK], F32R, tag="tp")
    nc.scalar.activation(pre[:], psc[:], AF.Prelu, bias=b1_sb[:, 0:1],
                         alpha=a1)
    psc2 = ps.tile([C, CHUNK], F32, tag="pscv")
    nc.tensor.matmul(psc2[:], _r(w2t_sb[:]), _r(pre[:]), start=True, stop=True)
    if out_ap is None:
        c = kpool.tile([C, CHUNK], F32R, tag="tc")
        out_ap = c[:]
    nc.scalar.activation(out_ap, psc2[:], AF.Prelu, bias=b2_sb[:, 0:1],
                         alpha=a2)
    return out_ap


def _build(a1: float, a2: float, loop_n: int = 1):
    nc = bacc.Bacc(None, target_bir_lowering=False)
    xr = nc.declare_dram_parameter("xr", [C, N], F32R, isOutput=False)
    xd = nc.declare_dram_parameter("xd", [C, N], F32R, isOutput=False)
    xdq = nc.declare_dram_parameter("xdq", [C, 26 * W], F32R, isOutput=False)
    w1t = nc.declare_dram_parameter("w1t", [C, 9 * C], F32R, isOutput=False)
    w2t = nc.declare_dram_parameter("w2t", [C, C], F32R, isOutput=False)
    wch1t = nc.declare_dram_parameter("wch1t", [C, CQ], F32R, isOutput=False)
    wch2t = nc.declare_dram_parameter("wch2t", [C, CQ], F32R, isOutput=False)
    wch3t = nc.declare_dram_parameter("wch3t", [C, CQ], F32R, isOutput=False)
    wch4t = nc.declare_dram_parameter("wch4t", [CQ, C], F32R, isOutput=False)
    b1 = nc.declare_dram_parameter("b1", [C, 2], F32, isOutput=False)
    b2 = nc.declare_dram_parameter("b2", [C, 2], F32, isOutput=False)
    zz = nc.declare_dram_parameter("zz", [C, PW], F32R, isOutput=False)
    kones = nc.declare_dram_parameter("kones", [128, NT], F32R, isOutput=False)
    out = nc.declare_dram_parameter("out", [C, NQ], F32, isOutput=True)

    with tile.TileContext(nc) as tc:
        with (
            tc.tile_pool(name="const", bufs=1) as cpool,
            tc.tile_pool(name="xpad", bufs=1) as xpool,
            tc.tile_pool(name="big", bufs=1) as bpool,
            tc.tile_pool(name="chunk", bufs=3) as kpool,
            tc.tile_pool(name="pt", bufs=3) as ptpool,
            tc.tile_pool(name="ep", bufs=2) as eppool,
            tc.tile_pool(name="ps_s", bufs=2, space="PSUM") as ps_s,
            tc.tile_pool(name="ps_g", bufs=2, space="PSUM") as ps_g,
            tc.tile_pool(name="ps_m", bufs=2, space="PSUM") as ps_m,
        ):
            # ---- constants ----
            w1t_sb = cpool.tile([C, 9 * C], F32R)
            nc.sync.dma_start(w1t_sb[:], w1t[:])
            w2t_sb = cpool.tile([C, C], F32R)
            nc.sync.dma_start(w2t_sb[:], w2t[:])
            wch1t_sb = cpool.tile([C, CQ], F32R)
            nc.sync.dma_start(wch1t_sb[:], wch1t[:])
            wch2t_sb = cpool.tile([C, CQ], F32R)
            nc.sync.dma_start(wch2t_sb[:], wch2t[:])
            wch3t_sb = cpool.tile([C, CQ], F32R)
            nc.sync.dma_start(wch3t_sb[:], wch3t[:])
            wch4t_sb = cpool.tile([CQ, C], F32R)
            nc.sync.dma_start(wch4t_sb[:], wch4t[:])
            b1_sb = cpool.tile([C, 2], F32)
            nc.sync.dma_start(b1_sb[:], b1[:])
            b2_sb = cpool.tile([C, 2], F32)
            nc.sync.dma_start(b2_sb[:], b2[:])
            ones_sb = cpool.tile([1, C], F32R)
            nc.sync.dma_start(ones_sb[:], kones[0:1, 0:C])

            import contextlib
            loop_cm = tc.For_i(0, loop_n, 1) if loop_n > 1 else \
                contextlib.nullcontext()
            with loop_cm:
                _body(nc, tc, locals())

    nc.finalize()
    return nc


def _body(nc, tc, env):
    (cpool, xpool, bpool, kpool, ptpool, eppool, ps_s, ps_g, ps_m) = (
        env[k] for k in ("cpool", "xpool", "bpool", "kpool", "ptpool",
                         "eppool", "ps_s", "ps_g", "ps_m"))
    (w1t_sb, w2t_sb, wch1t_sb, wch2t_sb, wch3t_sb, wch4t_sb, b1_sb, b2_sb,
     ones_sb) = (env[k] for k in ("w1t_sb", "w2t_sb", "wch1t_sb", "wch2t_sb",
                                  "wch3t_sb", "wch4t_sb", "b1_sb", "b2_sb",
                                  "ones_sb"))
    (xr, xd, xdq, out, a1, a2, zz, kones) = (env[k] for k in
                                  ("xr", "xd", "xdq", "out", "a1", "a2",
                                   "zz", "kones"))
    if True:
        if True:

            # ---- persistent intermediates ----
            d2_sb = bpool.tile([CQ, N], F32R)           # scores lhsT source
            c1aug = bpool.tile([128, NT, CQ + 1], F32R)  # c1 N-major + ones col
            d1q = bpool.tile([CQ, NQ], F32R)
            d0q = bpool.tile([C, NQ], F32R)
            nc.sync.dma_start(c1aug[:, :, CQ:CQ + 1], kones[:].unsqueeze(2))

            # ---- depth quarter (halo slab): d0q, d1q ----
            dq_slab = xpool.tile([C, 26, PW], F32R, tag="dqslab")
            nc.sync.dma_start(dq_slab[:, :, 0:1], zz[:, 0:26].unsqueeze(2))
            nc.sync.dma_start(dq_slab[:, :, PW - 1:PW], zz[:, 0:26].unsqueeze(2))
            nc.sync.dma_start(
                dq_slab[:, :, 1:W + 1],
                xdq[:].rearrange("c (r w) -> c r w", w=W),
            )
            for j in range(NQ // CHUNK):
                sl = slice(j * CHUNK, (j + 1) * CHUNK)
                _trunk_chunk(nc, tc, kpool, ps_m, dq_slab, 4 * j, w1t_sb,
                             w2t_sb, b1_sb, b2_sb, a1, a2, out_ap=d0q[:, sl])
                psq = ps_m.tile([CQ, CHUNK], F32, tag="pscv")
                nc.tensor.matmul(psq[:], _r(wch2t_sb[:]), _r(d0q[:, sl]),
                                 start=True, stop=True)
                nc.vector.tensor_copy(d1q[:, sl], psq[:])

            # ---- rgb trunk -> c1aug (N-major) ----
            slab = xpool.tile([C, PW, PW], F32R, tag="slab")
            for edge in (0, PW - 1):
                nc.sync.dma_start(slab[:, edge, :], zz[:])
                nc.sync.dma_start(slab[:, 1:PW - 1, edge:edge + 1],
                                  zz[:, 0:PW - 2].unsqueeze(2))
            xr_r = xr[:].rearrange("c (h w) -> c h w", w=W)
            for piece in range(4):
                rs = slice(piece * QROWS, (piece + 1) * QROWS)
                nc.sync.dma_start(slab[:, 1 + piece * QROWS:1 + (piece + 1) * QROWS, 1:W + 1],
                                  xr_r[:, rs, :])
            for j in range(N // CHUNK):
                c = _trunk_chunk(nc, tc, kpool, ps_m, slab, 4 * j, w1t_sb,
                                 w2t_sb, b1_sb, b2_sb, a1, a2)
                for i in range(3):
                    ti = 3 * j + i
                    psn = ps_m.tile([128, CQ], F32, tag="pscv")
                    nc.tensor.matmul(psn[:], _r(c[:, i * 128:(i + 1) * 128]),
                                     _r(wch1t_sb[:]), start=True, stop=True)
                    nc.vector.tensor_copy(c1aug[:, ti, 0:CQ], psn[:])

            # ---- depth trunk -> d2 (channel-major) ----
            slab2 = xpool.tile([C, PW, PW], F32R, tag="slab")
            for edge in (0, PW - 1):
                nc.sync.dma_start(slab2[:, edge, :], zz[:])
                nc.sync.dma_start(slab2[:, 1:PW - 1, edge:edge + 1],
                                  zz[:, 0:PW - 2].unsqueeze(2))
            xd_r = xd[:].rearrange("c (h w) -> c h w", w=W)
            for piece in range(4):
                rs = slice(piece * QROWS, (piece + 1) * QROWS)
                nc.sync.dma_start(slab2[:, 1 + piece * QROWS:1 + (piece + 1) * QROWS, 1:W + 1],
                                  xd_r[:, rs, :])
            for j in range(N // CHUNK):
                d = _trunk_chunk(nc, tc, kpool, ps_m, slab2, 4 * j, w1t_sb,
                                 w2t_sb, b1_sb, b2_sb, a1, a2)
                psd = ps_m.tile([CQ, CHUNK], F32, tag="pscv")
                nc.tensor.matmul(psd[:], _r(wch3t_sb[:]), _r(d),
                                 start=True, stop=True)
                nc.vector.tensor_copy(d2_sb[:, j * CHUNK:(j + 1) * CHUNK], psd[:])

            # ---- streaming attention over row blocks ----
            for (o, rb) in BLOCKS:
                ps_acc = ps_g.tile([CQ + 1, rb], F32, tag="psg")
                for u in range(NT // 2):
                    t0, t1 = 2 * u, 2 * u + 1
                    ps_sc = ps_s.tile([128, 2 * rb], F32, tag="pss")
                    nc.tensor.matmul(ps_sc[:, 0:rb],
                                     _r(d2_sb[:, t0 * 128:(t0 + 1) * 128]),
                                     _r(d1q[:, o:o + rb]), start=True, stop=True)
                    nc.tensor.matmul(ps_sc[:, rb:2 * rb],
                                     _r(d2_sb[:, t1 * 128:(t1 + 1) * 128]),
                                     _r(d1q[:, o:o + rb]), start=True, stop=True)
                    pT = ptpool.tile([128, 2 * rb], F32R, tag="pt")
                    nc.scalar.activation(pT[:], ps_sc[:], AF.Exp)
                    nc.tensor.matmul(ps_acc[:], _r(c1aug[:, t0, :]),
                                     _r(pT[:, 0:rb]),
                                     start=(t0 == 0), stop=False,
                                     skip_group_check=True)
                    nc.tensor.matmul(ps_acc[:], _r(c1aug[:, t1, :]),
                                     _r(pT[:, rb:2 * rb]),
                                     start=False, stop=(t1 == NT - 1),
                                     skip_group_check=True)
                g_sb = eppool.tile([CQ, rb], F32R, tag="gsb")
                nc.vector.tensor_copy(g_sb[:], ps_acc[0:CQ, :])
                sum_sb = eppool.tile([1, rb], F32R, tag="ssb")
                nc.vector.tensor_copy(sum_sb[:], ps_acc[CQ:CQ + 1, :])
                ps_b = ps_m.tile([C, rb], F32, tag="pscv")
                nc.tensor.matmul(ps_b[:], _r(ones_sb[:]), _r(sum_sb[:]),
                                 start=True, stop=True)
                rcp = eppool.tile([C, rb], F32, tag="rcp")
                nc.vector.reciprocal(rcp[:], ps_b[:])
                ps_o = ps_m.tile([C, rb], F32, tag="pscv")
                nc.tensor.matmul(ps_o[:], _r(wch4t_sb[:]), _r(g_sb[:]),
                                 start=True, stop=True)
                o1 = eppool.tile([C, rb], F32, tag="o1")
                nc.vector.tensor_mul(o1[:], ps_o[:], rcp[:])
                osb = eppool.tile([C, rb], F32, tag="osb")
                nc.vector.tensor_add(osb[:], o1[:], d0q[:, o:o + rb].bitcast(F32))
                nc.sync.dma_start(out[:, o:o + rb], osb[:])


def _prep_inputs(rgb, depth, w1, b1, a1, w2, b2, a2, wch1, wch2, wch3, wch4):
    rgb = np.asarray(rgb, np.float32)
    depth = np.asarray(depth, np.float32)
    # w1t[ci, (ky*3+kx)*C + co]
    w1t = np.ascontiguousarray(
        np.transpose(np.asarray(w1, np.float32), (1, 2, 3, 0)).reshape(C, 9 * C))
    w2t = np.ascontiguousarray(np.asarray(w2, np.float32)[:, :, 0, 0].T)
    wch1t = np.ascontiguousarray(np.asarray(wch1, np.float32)[:, :, 0, 0].T)
    wch2t = np.ascontiguousarray(np.asarray(wch2, np.float32)[:, :, 0, 0].T)
    wch3t = np.ascontiguousarray(np.asarray(wch3, np.float32)[:, :, 0, 0].T)
    wch4t = np.ascontiguousarray(np.asarray(wch4, np.float32)[:, :, 0, 0].T)
    a1f = float(np.asarray(a1)); a2f = float(np.asarray(a2))
    b1a = np.stack([np.asarray(b1, np.float32)] * 2, axis=1)
    b2a = np.stack([np.asarray(b2, np.float32)] * 2, axis=1)

    in_maps = []
    for core in range(8):
        s, q = divmod(core, 4)
        xdq = np.zeros((C, 26, W), np.float32)
        for r_slab in range(26):
            r_img = q * QROWS - 1 + r_slab
            if 0 <= r_img < H:
                xdq[:, r_slab, :] = depth[s, :, r_img, :]
        in_maps.append({
            "xr": np.ascontiguousarray(rgb[s].reshape(C, N)),
            "xd": np.ascontiguousarray(depth[s].reshape(C, N)),
            "xdq": np.ascontiguousarray(xdq.reshape(C, 26 * W)),
            "w1t": w1t, "w2t": w2t,
            "wch1t": wch1t, "wch2t": wch2t, "wch3t": wch3t, "wch4t": wch4t,
            "b1": np.ascontiguousarray(b1a), "b2": np.ascontiguousarray(b2a),
            "zz": np.zeros((C, PW), np.float32),
            "kones": np.ones((128, NT), np.float32),
        })
    return in_maps, (a1f, a2f)


def kernel(rgb, depth, w1, b1, a1, w2, b2, a2, wch1, wch2, wch3, wch4,
           _loop_n=1, **run_kwargs):
    in_maps, (a1f, a2f) = _prep_inputs(rgb, depth, w1, b1, a1, w2, b2, a2,
                                       wch1, wch2, wch3, wch4)
    key = (a1f, a2f, _loop_n)
    if key not in _cache:
        _cache[key] = _build(a1f, a2f, loop_n=_loop_n)
    nc = _cache[key]
    res = run_bass_kernel_spmd(nc, in_maps, list(range(8)), **run_kwargs)
    out_full = np.empty((2, C, H, W), np.float32)
    for core in range(8):
        s, q = divmod(core, 4)
        out_full[s, :, q * QROWS:(q + 1) * QROWS, :] = \
            res.results[core]["out"].reshape(C, QROWS, W)
    if run_kwargs:
        return out_full, res
    return out_full
